# revision 45
# baseline (speedup 1.0000x reference)
"""Trainium2 Bass kernel: BinarizedLinear  out = x @ (u < weight).T

Shapes (hardcoded): x [16384, 4096] f32, weight/u [512, 4096] f32,
out [16384, 512] f32.

Sharding: data-parallel over 8 NeuronCores — x sharded along batch
(2048 rows/core). weight/u are ALSO sharded (64 rows/core, delivered by
the host pre-transposed into the [p, kk, oc] k-plane layout): each core
binarizes only its own 1/8 of the weights and the 8 cores exchange the
resulting fp8 weightB via an on-device AllGather (0.26 MiB/core in, 2
MiB out). This removes the 16.8 MiB/core replicated f32 weight/u HBM
read of the first revision — the kernel is HBM-bandwidth-bound
(~345 GB/s/core measured, 358 GB/s limit), so bytes are the objective.

Per-core kernel (Tile framework), fp8_e4m3 DoubleRow pipeline:

  Phase A (once): load the pre-transposed w/u shard f32 (2.1 MiB,
  full-width [128, 2048] lines), binarize straight to fp8 {0,1} with a
  single full-width DVE is_lt, store the 0.26 MiB shard to a DRAM
  bounce, AllGather -> [8, 128, 32, 64] fp8 in (Shared) DRAM, read
  back + repack to the resident wbtp[p, kk, o] = wB[o, kk*128 + p]
  tile (2 MiB SBUF). The collective's latency is hidden: phase B's x
  loads + PE transposes run 6 groups deep before the first
  weight-gated matmul, and the readback+repack are emitted BEHIND the
  first 3 groups' copies so no engine queue stalls on them.

  Phase B (per 2-tile batch group): SWDGE cast-load x f32->bf16,
  transpose every [128, 128] sub-tile on the TENSOR engine
  (identity-matmul into bf16 PSUM) and copy-cast PSUM->SBUF to fp8
  xtp[p, kk, b] = x[b0+b, kk*128 + p] on DVE/Act alternately. Matmuls
  run in MatmulPerfMode.DoubleRow: each instruction contracts K=256
  into a [128, 512] f32 PSUM bank at 2x bf16 PE throughput. DVE/Act
  copy PSUM -> SBUF, stores ride the sync HWDGE queue.

Measured ablations (loop-diff, per 2048-row pass): x cast-loads alone
97 us (345 GB/s, ~96% of the HBM-per-core limit), loads+stores 109 us,
compute-only (transposes+matmuls+copies) 100 us; phase B alone 125 us
(overlap of the 109 us DMA stream with 100 us of PE work), full
recurring pass 137.7 us (loop-diff excludes only the AllGather's own
latency, which overlaps the x stream). First revision (replicated f32
weights, xbar weight transpose): 151.6 us. Attempts that did NOT work:
fp8 PSUM transpose outputs (verifier rejects), remote_dma_broadcast
peer-SBUF weight exchange (Tile's single-core scheduling sim cannot
see cross-core semaphore increments -> compile-time deadlock),
slot-major strided matmul rhs (hangs the device), reps-based whole-NEFF
timing (collectives cannot replay inside a hardware For_i loop; axon
RPC noise swamps un-looped timing).

Numerics: weightB is exact {0,1} (f32 compare on device); x is
quantized f32->bf16 (DMA cast) then fp8e4m3 (PSUM copy-cast), so each
output is a 4096-term dot of fp8-rounded x against exact binary
weights: rel err ~6e-4 vs the 2e-2 gate. Output stores are f32
(ob16=True stores bf16, rel err 1.8e-3, no longer faster).
"""

import numpy as np

from concourse import bass, bacc, mybir, tile
from concourse.bass_utils import run_bass_kernel_spmd

B, INUM, ONUM = 16384, 4096, 512
NCORES = 8
BLOC = B // NCORES   # 2048 batch rows per core
OLOC = ONUM // NCORES  # 64 weight rows per core
P = 128
NK = INUM // P      # 32 k-planes of 128
NK2 = NK // 2       # 16 DoubleRow contraction steps

F32 = mybir.dt.float32
BF16 = mybir.dt.bfloat16
FP8 = mybir.dt.float8e4
DR = mybir.MatmulPerfMode.DoubleRow

_CACHE = {}


def build(bloc=BLOC, gb=2, xn_bufs=3, xt_bufs=6, ob_bufs=4, ps_bufs=6,
          qt=8, prefetch=5, loop=None, shared_cc=True, x8=False,
          ob16=False, loop_pa=True, wbufs=2, rdx=False, pret=True,
          norepack=False, abl=None):
    """qt: k-planes per PE-transpose PSUM staging tile (1 bank at 8).

    loop: if set, wrap phase B in a For_i repeating it `loop` times
    (timing variant: same data each iteration, outputs overwritten).
    """
    nbt = bloc // P
    ngrp = nbt // gb
    nc = bacc.Bacc("TRN2", target_bir_lowering=False, debug=False,
                   num_devices=NCORES)
    pret = pret or rdx
    x_d = nc.dram_tensor("x", [bloc, INUM], F32, kind="ExternalInput")
    if pret:
        # host delivers each core's w/u shard pre-transposed to the
        # [p, kk, oc] plane layout (w[c*64+oc, kk*128+p] at [p, kk, oc])
        w_d = nc.dram_tensor("weight", [P, NK, OLOC], F32,
                             kind="ExternalInput")
        u_d = nc.dram_tensor("u", [P, NK, OLOC], F32,
                             kind="ExternalInput")
    else:
        w_d = nc.dram_tensor("weight", [OLOC, INUM], F32,
                             kind="ExternalInput")
        u_d = nc.dram_tensor("u", [OLOC, INUM], F32, kind="ExternalInput")
    ODT = BF16 if ob16 else F32
    o_d = nc.dram_tensor("out", [bloc, ONUM], ODT, kind="ExternalOutput")
    XDT = FP8 if x8 else BF16

    # DRAM views: x_v[g][p, j, i] = x[(g*gb + j)*P + p, i]
    x_v = x_d[:, :].rearrange("(g j p) i -> g p j i", g=ngrp, j=gb, p=P)
    o_v = o_d[:, :].rearrange("(t p) o -> t p o", t=nbt, p=P)

    def _vcopy(out, in_):
        return nc.vector.tensor_copy(out, in_)

    def _scopy(out, in_):
        return nc.scalar.copy(out, in_)

    engs = [_vcopy, _scopy]

    with tile.TileContext(nc) as tc:
        with (
            tc.tile_pool(name="wres", bufs=wbufs) as wres_pool,
            tc.tile_pool(name="ps", bufs=ps_bufs, space="PSUM") as ps_pool,
            tc.tile_pool(name="ident", bufs=1) as ident_pool,
            tc.tile_pool(name="pst", bufs=2, space="PSUM") as pst_pool,
            tc.tile_pool(name="wu", bufs=1) as wu_pool,
            tc.tile_pool(name="wt", bufs=1) as wt_pool,
            tc.tile_pool(name="ccd", bufs=1, space="DRAM") as ccd_pool,
            tc.tile_pool(name="xn", bufs=xn_bufs) as xn_pool,
            tc.tile_pool(name="xt", bufs=xt_bufs) as xt_pool,
            tc.tile_pool(name="ob", bufs=ob_bufs) as ob_pool,
        ):
            from concourse import masks
            ident = ident_pool.tile([P, P], XDT)
            masks.make_identity(nc, ident[:])
            ncopy = [0]

            # cc bounce buffers live outside the loop: the AllGather runs
            # exactly once per NEFF execution (NRT cannot replay a
            # collective inside a hardware loop).
            cc_in = ccd_pool.tile([P, NK, OLOC], FP8, tag="cci")
            cc_out = ccd_pool.tile(
                [NCORES, P, NK, OLOC], FP8, tag="cco",
                addr_space=("Shared" if shared_cc else "Local"))
            if rdx:
                # remote-DMA exchange: slot-major gathered weights, written
                # directly into SBUF by all 8 cores' broadcasts (slot s on
                # core r holds the shard of core r XOR s — the host gather
                # un-permutes the output column blocks). One static buffer:
                # peers write the compile-time symmetric address.
                wbtp2 = wres_pool.tile([P, NCORES, NK, OLOC], FP8,
                                       tag="wbtp2")
                rsem = nc.alloc_semaphore("wb_recv_sem")
                lsem = nc.alloc_semaphore("wb_sent_sem")

            def phase_a_pre():
                """w/u shard -> binarized, transposed fp8 in wshT8."""
                if pret:
                    # pre-transposed layout: one full-width DVE compare
                    wshT8 = wt_pool.tile([P, NK, OLOC], FP8, tag="wshT8")
                    w_t = wu_pool.tile([P, NK, OLOC], F32, tag="w")
                    u_t = wu_pool.tile([P, NK, OLOC], F32, tag="u")
                    nc.scalar.dma_start(out=w_t[:], in_=w_d[:, :])
                    nc.sync.dma_start(out=u_t[:], in_=u_d[:, :])
                    nc.vector.tensor_tensor(wshT8[:], u_t[:], w_t[:],
                                            op=mybir.AluOpType.is_lt)
                    if not rdx:
                        nc.sync.dma_start(out=cc_in[:], in_=wshT8[:])
                    return wshT8
                wshT8 = wt_pool.tile([P, NK, OLOC], FP8, tag="wshT8")
                if abl == "nowu":
                    nc.vector.memset(wshT8[:], 0.0)
                    nc.sync.dma_start(out=cc_in[:], in_=wshT8[:])
                    return wshT8
                w_t = wu_pool.tile([OLOC, INUM], F32, tag="w")
                u_t = wu_pool.tile([OLOC, INUM], F32, tag="u")
                wsh = wt_pool.tile([OLOC, INUM], BF16, tag="wsh")
                nc.gpsimd.dma_start(out=w_t[:], in_=w_d[:, :])
                nc.scalar.dma_start(out=u_t[:], in_=u_d[:, :])
                nc.vector.tensor_tensor(wsh[:], u_t[:], w_t[:],
                                        op=mybir.AluOpType.is_lt)
                if abl == "noxbar":
                    nc.vector.memset(wshT8[:], 0.0)
                else:
                    wshT = wt_pool.tile([P, NK, OLOC], BF16, tag="wshT")
                    # wshT[p, kk, oc] = wsh[oc, kk*128 + p]
                    nc.sync.dma_start(out=wshT[:], in_=wsh[:],
                                      transpose=True)
                    nc.vector.tensor_copy(wshT8[:], wshT[:])
                nc.sync.dma_start(out=cc_in[:], in_=wshT8[:])
                return wshT8

            def phase_a_rdx(wshT8):
                """Broadcast my fp8 shard into slot (my_id XOR k) of every
                core's wbtp2, self included. Descriptor generation first,
                then the all-entered barrier, then the trigger — peers'
                SBUF may only be written once every core is inside this
                kernel (prior execution done with the buffer)."""
                for k in range(NCORES):
                    rdests = [None] * NCORES
                    rdests[k] = (0, k)
                    nc.gpsimd.remote_dma_broadcast(
                        out_ap=wbtp2[:, k],
                        in_ap=wshT8[:],
                        remote_sem=rsem,
                        local_sem=lsem,
                        rdests=rdests)
                nc.gpsimd.bir_kernel_barrier_wait([list(range(NCORES))])
                nc.gpsimd.trigger_dma(count=None)

            def emit_cc():
                nc.gpsimd.collective_compute(
                    "AllGather",
                    mybir.AluOpType.bypass,
                    replica_groups=[list(range(NCORES))],
                    ins=[cc_in[:].opt()],
                    outs=[cc_out[:].opt()],
                )

            def phase_a_post():
                """Gathered fp8 weights -> resident wbtp[p, kk, o]."""
                wg = wres_pool.tile([P, NCORES, NK, OLOC], FP8, tag="wg")
                for c in range(NCORES):
                    q = nc.sync if c % 2 == 0 else nc.scalar
                    q.dma_start(out=wg[:, c], in_=cc_out[c])
                if norepack:
                    # matmuls read the slot-major tile directly (CC slot
                    # order IS absolute o-block order)
                    return wg
                wbtp = wres_pool.tile([P, NK, ONUM], FP8, tag="wbtp")
                for c in range(NCORES):
                    engs[c % 2](wbtp[:, :, c * OLOC:(c + 1) * OLOC],
                                wg[:, c])
                return wbtp

            def emit_fetch(g, xts):
                # xnb[p, j, i] = x[(g*gb + j)*P + p, i] cast on load
                xnb = xn_pool.tile([P, gb, INUM], XDT, tag="xn")
                nc.gpsimd.dma_start(out=xnb[:], in_=x_v[g])
                # PE transposes -> fp8 true-plane
                # xtp[p, j*NK + kk, b] = x[(g*gb+j)*P + b, kk*P + p]
                xtp = xt_pool.tile([P, gb * NK, P], FP8, tag="xtp")
                for j in range(gb):
                    for q in range(NK // qt):
                        pst = pst_pool.tile([P, qt, P], XDT, tag="pst")
                        for h in range(qt):
                            kk = q * qt + h
                            nc.tensor.transpose(
                                pst[:, h, :],
                                xnb[:, j, kk * P:(kk + 1) * P],
                                ident[:])
                        kk0 = j * NK + q * qt
                        e = engs[ncopy[0] % 2]
                        ncopy[0] += 1
                        e(xtp[:, kk0:kk0 + qt, :], pst[:])
                xts[g] = xtp

            def phase_b(wbtp, xts, start_g):
                # software-pipelined emission: group g+prefetch's
                # load+transpose are emitted before group g's matmuls
                for g in range(start_g, min(prefetch + 1, ngrp)):
                    emit_fetch(g, xts)
                if rdx:
                    # gate the PE on all 8 broadcast shards having landed
                    # (7 peers + self, 2 sem incs each)
                    nc.tensor.wait_ge(rsem, 2 * NCORES)
                for g in range(ngrp):
                    xtp = xts.pop(g)
                    for j in range(gb):
                        bt = g * gb + j
                        ps = ps_pool.tile([P, ONUM], F32, tag="ps")
                        for k2 in range(NK2):
                            kk0 = j * NK + 2 * k2
                            if rdx:
                                rhs = wbtp2[:, :, 2 * k2:2 * k2 + 2, :] \
                                    .transpose([0, 2, 1, 3])
                            elif norepack:
                                rhs = wbtp[:, :, 2 * k2:2 * k2 + 2, :] \
                                    .transpose([0, 2, 1, 3])
                            else:
                                rhs = wbtp[:, 2 * k2:2 * k2 + 2, :]
                            nc.tensor.matmul(
                                ps[:],
                                xtp[:, kk0:kk0 + 2, :],
                                rhs,
                                start=(k2 == 0),
                                stop=(k2 == NK2 - 1),
                                perf_mode=DR)
                        ob = ob_pool.tile([P, ONUM], ODT, tag="ob")
                        e = engs[ncopy[0] % 2]
                        ncopy[0] += 1
                        e(ob[:], ps[:])
                        nc.sync.dma_start(out=o_v[bt], in_=ob[:])
                    if g + prefetch + 1 < ngrp:
                        emit_fetch(g + prefetch + 1, xts)

            def run_pass(first, wbtp0=None):
                """One full pass. Emission order matters: the first few x
                prefetches go out before the CC trigger / readback+repack,
                so neither the gpsimd DMA queue nor the DVE/Act copy queues
                stall on weight-side dependencies (the repack waits on the
                previous iteration's last matmul via the wbtp buffer — it
                must sit BEHIND a few groups' xtp copies in engine order,
                not ahead of them)."""
                xts = {}
                pre_g = min(3, prefetch + 1, ngrp)
                wshT8 = phase_a_pre()
                for g in range(pre_g):
                    emit_fetch(g, xts)
                if rdx:
                    if first:
                        phase_a_rdx(wshT8)
                    wbtp = None
                else:
                    if first:
                        emit_cc()
                    wbtp = phase_a_post() if wbtp0 is None else wbtp0
                phase_b(wbtp, xts, start_g=pre_g)

            if loop is None:
                # real path: single pass, exchange included.
                run_pass(first=True)
            else:
                # timing path: the exchange (CC / remote broadcast) runs
                # once up front — NRT cannot replay a collective inside a
                # hardware loop, and re-broadcasting per iteration would
                # race across iterations. Its latency overlaps phase B's x
                # stream in the real path. Every other recurring cost loops
                # on-device.
                if rdx:
                    wshT8 = phase_a_pre()
                    phase_a_rdx(wshT8)
                    wbtp0 = None
                else:
                    phase_a_pre()
                    emit_cc()
                    wbtp0 = None if loop_pa else phase_a_post()
                with tc.For_i(0, loop, 1):
                    run_pass(first=False, wbtp0=wbtp0)

    nc.compile()
    return nc


def _make_exec(nc):
    """Build a jitted shard_map executable over the 8 cores (mirrors
    bass2jax.run_bass_via_pjrt's multi-core path, without donation so the
    same device buffers can be re-executed for timing)."""
    import jax
    from jax.sharding import Mesh, PartitionSpec
    from jax.experimental.shard_map import shard_map
    from concourse import bass2jax

    bass2jax.install_neuronx_cc_hook()
    partition_name = (nc.partition_id_tensor.name
                      if nc.partition_id_tensor else None)
    in_names, out_names, out_avals = [], [], []
    for alloc in nc.m.functions[0].allocations:
        if not isinstance(alloc, mybir.MemoryLocationSet):
            continue
        name = alloc.memorylocations[0].name
        if alloc.kind == "ExternalInput":
            if name != partition_name:
                in_names.append(name)
        elif alloc.kind == "ExternalOutput":
            out_names.append(name)
            out_avals.append(jax.core.ShapedArray(
                tuple(alloc.tensor_shape), mybir.dt.np(alloc.dtype)))
    n_params = len(in_names)
    all_names = in_names + out_names
    if partition_name is not None:
        all_names = all_names + [partition_name]

    def _body(*args):
        operands = list(args)
        if partition_name is not None:
            operands.append(bass2jax.partition_id_tensor())
        return tuple(bass2jax._bass_exec_p.bind(
            *operands,
            out_avals=tuple(out_avals),
            in_names=tuple(all_names),
            out_names=tuple(out_names),
            lowering_input_output_aliases=(),
            sim_require_finite=True,
            sim_require_nnan=True,
            nc=nc,
        ))

    devices = jax.devices()[:NCORES]
    mesh = Mesh(np.asarray(devices), ("core",))

    def make_fn(reps):
        def _rep_body(*args):
            outs = None
            for _ in range(reps):
                outs = _body(*args)   # effectful primitive: not CSE'd
            return outs
        return jax.jit(
            shard_map(_rep_body, mesh=mesh,
                      in_specs=(PartitionSpec("core"),) * (n_params + len(out_names)),
                      out_specs=(PartitionSpec("core"),) * len(out_names),
                      check_rep=False),
            keep_unused=True,
        )

    return make_fn, mesh, in_names[:n_params], out_names, out_avals


def _prep_wu_rdx(t, c):
    """Core c's 64-row shard of weight/u, pre-transposed to the
    [p, kk, oc] plane layout the pret/rdx kernel loads directly."""
    s = t[c * OLOC:(c + 1) * OLOC]                    # [64, 4096]
    return np.ascontiguousarray(
        s.T.reshape(NK, P, OLOC).transpose(1, 0, 2))


def _unpermute_rdx(out_c, c):
    """rdx stores output column block a at slot c XOR a; undo that."""
    perm = [c ^ a for a in range(NCORES)]
    return out_c.reshape(out_c.shape[0], NCORES, OLOC)[:, perm] \
        .reshape(out_c.shape[0], ONUM)


def bench(x, weight, u, r_lo=32, r_hi=512, iters=6, **build_kw):
    """Measure real device time for one kernel execution.

    The axon RPC jitter (tens of ms) swamps a single execution, and
    multiple identical bass_exec calls in one program are too noisy to
    difference. So we build two NEFF variants whose WHOLE body (phase A
    incl. the AllGather + phase B) repeats in an on-device For_i loop
    (r_lo and r_hi iterations) and difference the wall-clock minima:
    (t_hi - t_lo)/(r_hi - r_lo) is one full kernel pass of device time —
    no separate phase-A estimate needed."""
    import time
    import jax
    from jax.sharding import NamedSharding, PartitionSpec

    if build_kw.get("rdx") or build_kw.get("pret", True):
        wcat = np.concatenate(
            [_prep_wu_rdx(weight, c) for c in range(NCORES)], axis=0)
        ucat = np.concatenate(
            [_prep_wu_rdx(u, c) for c in range(NCORES)], axis=0)
    else:
        wcat = np.ascontiguousarray(weight, dtype=np.float32)
        ucat = np.ascontiguousarray(u, dtype=np.float32)
    concat = {
        "x": np.ascontiguousarray(x, dtype=np.float32),
        "weight": wcat,
        "u": ucat,
    }

    def run_variant(r):
        nc = build(loop=r, **build_kw)
        make_fn, mesh, in_names, out_names, out_avals = _make_exec(nc)
        sh = NamedSharding(mesh, PartitionSpec("core"))
        args = [jax.device_put(concat[n], sh) for n in in_names]
        zeros = [
            jax.device_put(
                np.zeros((NCORES * a.shape[0], *a.shape[1:]), a.dtype), sh)
            for a in out_avals
        ]
        fn = make_fn(1)
        jax.block_until_ready(fn(*args, *zeros))    # compile + warm
        times = []
        for _ in range(iters):
            t0 = time.perf_counter()
            jax.block_until_ready(fn(*args, *zeros))
            times.append(time.perf_counter() - t0)
        # median, not min: a rare async-return artifact can produce a
        # physically impossible minimum (seen once after a worker restart)
        times.sort()
        return times[len(times) // 2]

    t_lo = run_variant(r_lo)
    t_hi = run_variant(r_hi)
    pass_ns = (t_hi - t_lo) / (r_hi - r_lo) * 1e9
    print(f"bench: loop{r_lo}={t_lo*1e3:.1f}ms loop{r_hi}={t_hi*1e3:.1f}ms "
          f"-> full pass {pass_ns/1e3:.1f}us")
    return pass_ns


KERNEL_KW = {}


def kernel(x, weight, u):
    x = np.ascontiguousarray(np.asarray(x), dtype=np.float32)
    weight = np.ascontiguousarray(np.asarray(weight), dtype=np.float32)
    u = np.ascontiguousarray(np.asarray(u), dtype=np.float32)
    assert x.shape == (B, INUM) and weight.shape == (ONUM, INUM)
    rdx = bool(KERNEL_KW.get("rdx"))
    pret = rdx or bool(KERNEL_KW.get("pret", True))

    key = tuple(sorted(KERNEL_KW.items()))
    nc = _CACHE.get(key)
    if nc is None:
        nc = _CACHE[key] = build(**KERNEL_KW)

    if pret:
        in_maps = [
            {"x": x[c * BLOC:(c + 1) * BLOC],
             "weight": _prep_wu_rdx(weight, c),
             "u": _prep_wu_rdx(u, c)}
            for c in range(NCORES)
        ]
    else:
        in_maps = [
            {"x": x[c * BLOC:(c + 1) * BLOC],
             "weight": weight[c * OLOC:(c + 1) * OLOC],
             "u": u[c * OLOC:(c + 1) * OLOC]}
            for c in range(NCORES)
        ]
    res = run_bass_kernel_spmd(nc, in_maps, list(range(NCORES)))
    outs = []
    for c in range(NCORES):
        out_c = np.asarray(res.results[c]["out"])
        if out_c.dtype != np.float32:   # ob16 builds store bf16
            out_c = out_c.astype(np.float32)
        if rdx:
            out_c = _unpermute_rdx(out_c, c)
        outs.append(out_c)
    return np.concatenate(outs, axis=0)


# revision 50
# speedup vs baseline: 1.0236x; 1.0236x over previous
"""Trainium2 Bass kernel: BinarizedLinear  out = x @ (u < weight).T

Shapes (hardcoded): x [16384, 4096] f32, weight/u [512, 4096] f32,
out [16384, 512] f32.

Sharding: data-parallel over 8 NeuronCores — x sharded along batch
(2048 rows/core). weight/u are ALSO sharded (64 rows/core, delivered by
the host pre-transposed into the [p, kk, oc] k-plane layout): each core
binarizes only its own 1/8 of the weights and the 8 cores exchange the
resulting fp8 weightB via an on-device AllGather (0.26 MiB/core in, 2
MiB out). This removes the 16.8 MiB/core replicated f32 weight/u HBM
read of the first revision — the kernel is HBM-bandwidth-bound
(~345 GB/s/core measured, 358 GB/s limit), so bytes are the objective.

Per-core kernel (Tile framework), fp8_e4m3 DoubleRow pipeline:

  Phase A (once): load the pre-transposed w/u shard f32 (2.1 MiB,
  full-width [128, 2048] lines), binarize straight to fp8 {0,1} with a
  single full-width DVE is_lt, store the 0.26 MiB shard to a DRAM
  bounce, AllGather -> [8, 128, 32, 64] fp8 in (Shared) DRAM, read
  back + repack to the resident wbtp[p, kk, o] = wB[o, kk*128 + p]
  tile (2 MiB SBUF). The collective's latency is hidden: phase B's x
  loads + PE transposes run 6 groups deep before the first
  weight-gated matmul, and the readback+repack are emitted BEHIND the
  first 3 groups' copies so no engine queue stalls on them.

  Phase B (per 128-row batch tile): SWDGE cast-load x f32->bf16, one
  2 MiB DMA + one SBUF tile per 128-row tile (split_x — transposes
  gate on 2 MiB landing, not 4, and the xn ring recycles at tile
  granularity; measured ~3 us/pass better than group-granularity
  loads). Transpose every [128, 128] sub-tile on the TENSOR engine
  (identity-matmul into bf16 PSUM) and copy-cast PSUM->SBUF to fp8
  xtp[p, kk, b] = x[b0+b, kk*128 + p] on DVE/Act alternately. Matmuls
  run in MatmulPerfMode.DoubleRow: each instruction contracts K=256
  into a [128, 512] f32 PSUM bank at 2x bf16 PE throughput. DVE/Act
  copy PSUM -> SBUF, stores ride the sync HWDGE queue.

Measured ablations (loop-diff, per 2048-row pass): x cast-loads alone
97 us (345 GB/s, ~96% of the HBM-per-core limit), loads+stores 109 us,
compute-only (transposes+matmuls+copies) 100 us; phase B alone 125 us
(overlap of the 109 us DMA stream with 100 us of PE work), full
recurring pass 136 us (loop-diff excludes only the AllGather's own
latency, which overlaps the x stream). First revision (replicated f32
weights, xbar weight transpose): 151.6 us. Attempts that did NOT work:
fp8 PSUM transpose outputs (verifier rejects), remote_dma_broadcast
peer-SBUF weight exchange (Tile's single-core scheduling sim cannot
see cross-core semaphore increments -> compile-time deadlock),
slot-major strided matmul rhs (hangs the device), reps-based whole-NEFF
timing (collectives cannot replay inside a hardware For_i loop; axon
RPC noise swamps un-looped timing).

Numerics: weightB is exact {0,1} (f32 compare on device); x is
quantized f32->bf16 (DMA cast) then fp8e4m3 (PSUM copy-cast), so each
output is a 4096-term dot of fp8-rounded x against exact binary
weights: rel err ~6e-4 vs the 2e-2 gate. Output stores are f32
(ob16=True stores bf16, rel err 1.8e-3, no longer faster).
"""

import numpy as np

from concourse import bass, bacc, mybir, tile
from concourse.bass_utils import run_bass_kernel_spmd

B, INUM, ONUM = 16384, 4096, 512
NCORES = 8
BLOC = B // NCORES   # 2048 batch rows per core
OLOC = ONUM // NCORES  # 64 weight rows per core
P = 128
NK = INUM // P      # 32 k-planes of 128
NK2 = NK // 2       # 16 DoubleRow contraction steps

F32 = mybir.dt.float32
BF16 = mybir.dt.bfloat16
FP8 = mybir.dt.float8e4
DR = mybir.MatmulPerfMode.DoubleRow

_CACHE = {}


def build(bloc=BLOC, gb=2, xn_bufs=8, xt_bufs=6, ob_bufs=4, ps_bufs=6,
          qt=8, prefetch=5, loop=None, shared_cc=True, x8=False,
          ob16=False, loop_pa=True, wbufs=2, rdx=False, pret=True,
          norepack=False, split_x=True, abl=None):
    """qt: k-planes per PE-transpose PSUM staging tile (1 bank at 8).

    loop: if set, wrap phase B in a For_i repeating it `loop` times
    (timing variant: same data each iteration, outputs overwritten).
    """
    nbt = bloc // P
    ngrp = nbt // gb
    nc = bacc.Bacc("TRN2", target_bir_lowering=False, debug=False,
                   num_devices=NCORES)
    pret = pret or rdx
    x_d = nc.dram_tensor("x", [bloc, INUM], F32, kind="ExternalInput")
    if pret:
        # host delivers each core's w/u shard pre-transposed to the
        # [p, kk, oc] plane layout (w[c*64+oc, kk*128+p] at [p, kk, oc])
        w_d = nc.dram_tensor("weight", [P, NK, OLOC], F32,
                             kind="ExternalInput")
        u_d = nc.dram_tensor("u", [P, NK, OLOC], F32,
                             kind="ExternalInput")
    else:
        w_d = nc.dram_tensor("weight", [OLOC, INUM], F32,
                             kind="ExternalInput")
        u_d = nc.dram_tensor("u", [OLOC, INUM], F32, kind="ExternalInput")
    ODT = BF16 if ob16 else F32
    o_d = nc.dram_tensor("out", [bloc, ONUM], ODT, kind="ExternalOutput")
    XDT = FP8 if x8 else BF16

    # DRAM views: x_v[g][p, j, i] = x[(g*gb + j)*P + p, i]
    x_v = x_d[:, :].rearrange("(g j p) i -> g p j i", g=ngrp, j=gb, p=P)
    o_v = o_d[:, :].rearrange("(t p) o -> t p o", t=nbt, p=P)

    def _vcopy(out, in_):
        return nc.vector.tensor_copy(out, in_)

    def _scopy(out, in_):
        return nc.scalar.copy(out, in_)

    engs = [_vcopy, _scopy]

    with tile.TileContext(nc) as tc:
        with (
            tc.tile_pool(name="wres", bufs=wbufs) as wres_pool,
            tc.tile_pool(name="ps", bufs=ps_bufs, space="PSUM") as ps_pool,
            tc.tile_pool(name="ident", bufs=1) as ident_pool,
            tc.tile_pool(name="pst", bufs=2, space="PSUM") as pst_pool,
            tc.tile_pool(name="wu", bufs=1) as wu_pool,
            tc.tile_pool(name="wt", bufs=1) as wt_pool,
            tc.tile_pool(name="ccd", bufs=1, space="DRAM") as ccd_pool,
            tc.tile_pool(name="xn", bufs=xn_bufs) as xn_pool,
            tc.tile_pool(name="xt", bufs=xt_bufs) as xt_pool,
            tc.tile_pool(name="ob", bufs=ob_bufs) as ob_pool,
        ):
            from concourse import masks
            ident = ident_pool.tile([P, P], XDT)
            masks.make_identity(nc, ident[:])
            ncopy = [0]

            # cc bounce buffers live outside the loop: the AllGather runs
            # exactly once per NEFF execution (NRT cannot replay a
            # collective inside a hardware loop).
            cc_in = ccd_pool.tile([P, NK, OLOC], FP8, tag="cci")
            cc_out = ccd_pool.tile(
                [NCORES, P, NK, OLOC], FP8, tag="cco",
                addr_space=("Shared" if shared_cc else "Local"))
            if rdx:
                # remote-DMA exchange: slot-major gathered weights, written
                # directly into SBUF by all 8 cores' broadcasts (slot s on
                # core r holds the shard of core r XOR s — the host gather
                # un-permutes the output column blocks). One static buffer:
                # peers write the compile-time symmetric address.
                wbtp2 = wres_pool.tile([P, NCORES, NK, OLOC], FP8,
                                       tag="wbtp2")
                rsem = nc.alloc_semaphore("wb_recv_sem")
                lsem = nc.alloc_semaphore("wb_sent_sem")

            def phase_a_pre():
                """w/u shard -> binarized, transposed fp8 in wshT8."""
                if pret:
                    # pre-transposed layout: one full-width DVE compare
                    wshT8 = wt_pool.tile([P, NK, OLOC], FP8, tag="wshT8")
                    w_t = wu_pool.tile([P, NK, OLOC], F32, tag="w")
                    u_t = wu_pool.tile([P, NK, OLOC], F32, tag="u")
                    nc.scalar.dma_start(out=w_t[:], in_=w_d[:, :])
                    nc.sync.dma_start(out=u_t[:], in_=u_d[:, :])
                    nc.vector.tensor_tensor(wshT8[:], u_t[:], w_t[:],
                                            op=mybir.AluOpType.is_lt)
                    if not rdx:
                        nc.sync.dma_start(out=cc_in[:], in_=wshT8[:])
                    return wshT8
                wshT8 = wt_pool.tile([P, NK, OLOC], FP8, tag="wshT8")
                if abl == "nowu":
                    nc.vector.memset(wshT8[:], 0.0)
                    nc.sync.dma_start(out=cc_in[:], in_=wshT8[:])
                    return wshT8
                w_t = wu_pool.tile([OLOC, INUM], F32, tag="w")
                u_t = wu_pool.tile([OLOC, INUM], F32, tag="u")
                wsh = wt_pool.tile([OLOC, INUM], BF16, tag="wsh")
                nc.gpsimd.dma_start(out=w_t[:], in_=w_d[:, :])
                nc.scalar.dma_start(out=u_t[:], in_=u_d[:, :])
                nc.vector.tensor_tensor(wsh[:], u_t[:], w_t[:],
                                        op=mybir.AluOpType.is_lt)
                if abl == "noxbar":
                    nc.vector.memset(wshT8[:], 0.0)
                else:
                    wshT = wt_pool.tile([P, NK, OLOC], BF16, tag="wshT")
                    # wshT[p, kk, oc] = wsh[oc, kk*128 + p]
                    nc.sync.dma_start(out=wshT[:], in_=wsh[:],
                                      transpose=True)
                    nc.vector.tensor_copy(wshT8[:], wshT[:])
                nc.sync.dma_start(out=cc_in[:], in_=wshT8[:])
                return wshT8

            def phase_a_rdx(wshT8):
                """Broadcast my fp8 shard into slot (my_id XOR k) of every
                core's wbtp2, self included. Descriptor generation first,
                then the all-entered barrier, then the trigger — peers'
                SBUF may only be written once every core is inside this
                kernel (prior execution done with the buffer)."""
                for k in range(NCORES):
                    rdests = [None] * NCORES
                    rdests[k] = (0, k)
                    nc.gpsimd.remote_dma_broadcast(
                        out_ap=wbtp2[:, k],
                        in_ap=wshT8[:],
                        remote_sem=rsem,
                        local_sem=lsem,
                        rdests=rdests)
                nc.gpsimd.bir_kernel_barrier_wait([list(range(NCORES))])
                nc.gpsimd.trigger_dma(count=None)

            def emit_cc():
                nc.gpsimd.collective_compute(
                    "AllGather",
                    mybir.AluOpType.bypass,
                    replica_groups=[list(range(NCORES))],
                    ins=[cc_in[:].opt()],
                    outs=[cc_out[:].opt()],
                )

            def phase_a_post():
                """Gathered fp8 weights -> resident wbtp[p, kk, o]."""
                wg = wt_pool.tile([P, NCORES, NK, OLOC], FP8, tag="wg")
                for c in range(NCORES):
                    q = nc.sync if c % 2 == 0 else nc.scalar
                    q.dma_start(out=wg[:, c], in_=cc_out[c])
                if norepack:
                    # matmuls read the slot-major tile directly (CC slot
                    # order IS absolute o-block order)
                    return wg
                wbtp = wres_pool.tile([P, NK, ONUM], FP8, tag="wbtp")
                for c in range(NCORES):
                    engs[c % 2](wbtp[:, :, c * OLOC:(c + 1) * OLOC],
                                wg[:, c])
                return wbtp

            def emit_fetch(g, xts):
                # xnb[p, j, i] = x[(g*gb + j)*P + p, i] cast on load.
                # split_x: one DMA + one SBUF tile per 128-row tile j, so
                # transposes of tile j gate on 2 MiB landing, not 4 MiB,
                # and the xn ring recycles at tile granularity.
                if split_x:
                    xnjs = []
                    for j in range(gb):
                        xnj = xn_pool.tile([P, INUM], XDT, tag="xn")
                        nc.gpsimd.dma_start(out=xnj[:], in_=x_v[g][:, j])
                        xnjs.append(xnj)
                else:
                    xnb = xn_pool.tile([P, gb, INUM], XDT, tag="xn")
                    nc.gpsimd.dma_start(out=xnb[:], in_=x_v[g])
                # PE transposes -> fp8 true-plane
                # xtp[p, j*NK + kk, b] = x[(g*gb+j)*P + b, kk*P + p]
                xtp = xt_pool.tile([P, gb * NK, P], FP8, tag="xtp")
                for j in range(gb):
                    src = xnjs[j] if split_x else None
                    for q in range(NK // qt):
                        pst = pst_pool.tile([P, qt, P], XDT, tag="pst")
                        for h in range(qt):
                            kk = q * qt + h
                            in_ = (src[:, kk * P:(kk + 1) * P] if split_x
                                   else xnb[:, j, kk * P:(kk + 1) * P])
                            nc.tensor.transpose(pst[:, h, :], in_, ident[:])
                        kk0 = j * NK + q * qt
                        e = engs[ncopy[0] % 2]
                        ncopy[0] += 1
                        e(xtp[:, kk0:kk0 + qt, :], pst[:])
                xts[g] = xtp

            def phase_b(wbtp, xts, start_g):
                # software-pipelined emission: group g+prefetch's
                # load+transpose are emitted before group g's matmuls
                for g in range(start_g, min(prefetch + 1, ngrp)):
                    emit_fetch(g, xts)
                if rdx:
                    # gate the PE on all 8 broadcast shards having landed
                    # (7 peers + self, 2 sem incs each)
                    nc.tensor.wait_ge(rsem, 2 * NCORES)
                for g in range(ngrp):
                    xtp = xts.pop(g)
                    for j in range(gb):
                        bt = g * gb + j
                        ps = ps_pool.tile([P, ONUM], F32, tag="ps")
                        for k2 in range(NK2):
                            kk0 = j * NK + 2 * k2
                            if rdx:
                                rhs = wbtp2[:, :, 2 * k2:2 * k2 + 2, :] \
                                    .transpose([0, 2, 1, 3])
                            elif norepack:
                                rhs = wbtp[:, :, 2 * k2:2 * k2 + 2, :] \
                                    .transpose([0, 2, 1, 3])
                            else:
                                rhs = wbtp[:, 2 * k2:2 * k2 + 2, :]
                            nc.tensor.matmul(
                                ps[:],
                                xtp[:, kk0:kk0 + 2, :],
                                rhs,
                                start=(k2 == 0),
                                stop=(k2 == NK2 - 1),
                                perf_mode=DR)
                        ob = ob_pool.tile([P, ONUM], ODT, tag="ob")
                        e = engs[ncopy[0] % 2]
                        ncopy[0] += 1
                        e(ob[:], ps[:])
                        nc.sync.dma_start(out=o_v[bt], in_=ob[:])
                    if g + prefetch + 1 < ngrp:
                        emit_fetch(g + prefetch + 1, xts)

            def run_pass(first, wbtp0=None):
                """One full pass. Emission order matters: the first few x
                prefetches go out before the CC trigger / readback+repack,
                so neither the gpsimd DMA queue nor the DVE/Act copy queues
                stall on weight-side dependencies (the repack waits on the
                previous iteration's last matmul via the wbtp buffer — it
                must sit BEHIND a few groups' xtp copies in engine order,
                not ahead of them)."""
                xts = {}
                pre_g = min(3, prefetch + 1, ngrp)
                wshT8 = phase_a_pre()
                for g in range(pre_g):
                    emit_fetch(g, xts)
                if rdx:
                    if first:
                        phase_a_rdx(wshT8)
                    wbtp = None
                else:
                    if first:
                        emit_cc()
                    wbtp = phase_a_post() if wbtp0 is None else wbtp0
                phase_b(wbtp, xts, start_g=pre_g)

            if loop is None:
                # real path: single pass, exchange included.
                run_pass(first=True)
            else:
                # timing path: the exchange (CC / remote broadcast) runs
                # once up front — NRT cannot replay a collective inside a
                # hardware loop, and re-broadcasting per iteration would
                # race across iterations. Its latency overlaps phase B's x
                # stream in the real path. Every other recurring cost loops
                # on-device.
                if rdx:
                    wshT8 = phase_a_pre()
                    phase_a_rdx(wshT8)
                    wbtp0 = None
                else:
                    phase_a_pre()
                    emit_cc()
                    wbtp0 = None if loop_pa else phase_a_post()
                with tc.For_i(0, loop, 1):
                    run_pass(first=False, wbtp0=wbtp0)

    nc.compile()
    return nc


def _make_exec(nc):
    """Build a jitted shard_map executable over the 8 cores (mirrors
    bass2jax.run_bass_via_pjrt's multi-core path, without donation so the
    same device buffers can be re-executed for timing)."""
    import jax
    from jax.sharding import Mesh, PartitionSpec
    from jax.experimental.shard_map import shard_map
    from concourse import bass2jax

    bass2jax.install_neuronx_cc_hook()
    partition_name = (nc.partition_id_tensor.name
                      if nc.partition_id_tensor else None)
    in_names, out_names, out_avals = [], [], []
    for alloc in nc.m.functions[0].allocations:
        if not isinstance(alloc, mybir.MemoryLocationSet):
            continue
        name = alloc.memorylocations[0].name
        if alloc.kind == "ExternalInput":
            if name != partition_name:
                in_names.append(name)
        elif alloc.kind == "ExternalOutput":
            out_names.append(name)
            out_avals.append(jax.core.ShapedArray(
                tuple(alloc.tensor_shape), mybir.dt.np(alloc.dtype)))
    n_params = len(in_names)
    all_names = in_names + out_names
    if partition_name is not None:
        all_names = all_names + [partition_name]

    def _body(*args):
        operands = list(args)
        if partition_name is not None:
            operands.append(bass2jax.partition_id_tensor())
        return tuple(bass2jax._bass_exec_p.bind(
            *operands,
            out_avals=tuple(out_avals),
            in_names=tuple(all_names),
            out_names=tuple(out_names),
            lowering_input_output_aliases=(),
            sim_require_finite=True,
            sim_require_nnan=True,
            nc=nc,
        ))

    devices = jax.devices()[:NCORES]
    mesh = Mesh(np.asarray(devices), ("core",))

    def make_fn(reps):
        def _rep_body(*args):
            outs = None
            for _ in range(reps):
                outs = _body(*args)   # effectful primitive: not CSE'd
            return outs
        return jax.jit(
            shard_map(_rep_body, mesh=mesh,
                      in_specs=(PartitionSpec("core"),) * (n_params + len(out_names)),
                      out_specs=(PartitionSpec("core"),) * len(out_names),
                      check_rep=False),
            keep_unused=True,
        )

    return make_fn, mesh, in_names[:n_params], out_names, out_avals


def _prep_wu_rdx(t, c):
    """Core c's 64-row shard of weight/u, pre-transposed to the
    [p, kk, oc] plane layout the pret/rdx kernel loads directly."""
    s = t[c * OLOC:(c + 1) * OLOC]                    # [64, 4096]
    return np.ascontiguousarray(
        s.T.reshape(NK, P, OLOC).transpose(1, 0, 2))


def _unpermute_rdx(out_c, c):
    """rdx stores output column block a at slot c XOR a; undo that."""
    perm = [c ^ a for a in range(NCORES)]
    return out_c.reshape(out_c.shape[0], NCORES, OLOC)[:, perm] \
        .reshape(out_c.shape[0], ONUM)


def bench(x, weight, u, r_lo=32, r_hi=512, iters=6, **build_kw):
    """Measure real device time for one kernel execution.

    The axon RPC jitter (tens of ms) swamps a single execution, and
    multiple identical bass_exec calls in one program are too noisy to
    difference. So we build two NEFF variants whose WHOLE body (phase A
    incl. the AllGather + phase B) repeats in an on-device For_i loop
    (r_lo and r_hi iterations) and difference the wall-clock minima:
    (t_hi - t_lo)/(r_hi - r_lo) is one full kernel pass of device time —
    no separate phase-A estimate needed."""
    import time
    import jax
    from jax.sharding import NamedSharding, PartitionSpec

    if build_kw.get("rdx") or build_kw.get("pret", True):
        wcat = np.concatenate(
            [_prep_wu_rdx(weight, c) for c in range(NCORES)], axis=0)
        ucat = np.concatenate(
            [_prep_wu_rdx(u, c) for c in range(NCORES)], axis=0)
    else:
        wcat = np.ascontiguousarray(weight, dtype=np.float32)
        ucat = np.ascontiguousarray(u, dtype=np.float32)
    concat = {
        "x": np.ascontiguousarray(x, dtype=np.float32),
        "weight": wcat,
        "u": ucat,
    }

    def run_variant(r):
        nc = build(loop=r, **build_kw)
        make_fn, mesh, in_names, out_names, out_avals = _make_exec(nc)
        sh = NamedSharding(mesh, PartitionSpec("core"))
        args = [jax.device_put(concat[n], sh) for n in in_names]
        zeros = [
            jax.device_put(
                np.zeros((NCORES * a.shape[0], *a.shape[1:]), a.dtype), sh)
            for a in out_avals
        ]
        fn = make_fn(1)
        jax.block_until_ready(fn(*args, *zeros))    # compile + warm
        times = []
        for _ in range(iters):
            t0 = time.perf_counter()
            jax.block_until_ready(fn(*args, *zeros))
            times.append(time.perf_counter() - t0)
        # median, not min: a rare async-return artifact can produce a
        # physically impossible minimum (seen once after a worker restart)
        times.sort()
        return times[len(times) // 2]

    t_lo = run_variant(r_lo)
    t_hi = run_variant(r_hi)
    pass_ns = (t_hi - t_lo) / (r_hi - r_lo) * 1e9
    print(f"bench: loop{r_lo}={t_lo*1e3:.1f}ms loop{r_hi}={t_hi*1e3:.1f}ms "
          f"-> full pass {pass_ns/1e3:.1f}us")
    return pass_ns


KERNEL_KW = {}


def kernel(x, weight, u):
    x = np.ascontiguousarray(np.asarray(x), dtype=np.float32)
    weight = np.ascontiguousarray(np.asarray(weight), dtype=np.float32)
    u = np.ascontiguousarray(np.asarray(u), dtype=np.float32)
    assert x.shape == (B, INUM) and weight.shape == (ONUM, INUM)
    rdx = bool(KERNEL_KW.get("rdx"))
    pret = rdx or bool(KERNEL_KW.get("pret", True))

    key = tuple(sorted(KERNEL_KW.items()))
    nc = _CACHE.get(key)
    if nc is None:
        nc = _CACHE[key] = build(**KERNEL_KW)

    if pret:
        in_maps = [
            {"x": x[c * BLOC:(c + 1) * BLOC],
             "weight": _prep_wu_rdx(weight, c),
             "u": _prep_wu_rdx(u, c)}
            for c in range(NCORES)
        ]
    else:
        in_maps = [
            {"x": x[c * BLOC:(c + 1) * BLOC],
             "weight": weight[c * OLOC:(c + 1) * OLOC],
             "u": u[c * OLOC:(c + 1) * OLOC]}
            for c in range(NCORES)
        ]
    res = run_bass_kernel_spmd(nc, in_maps, list(range(NCORES)))
    outs = []
    for c in range(NCORES):
        out_c = np.asarray(res.results[c]["out"])
        if out_c.dtype != np.float32:   # ob16 builds store bf16
            out_c = out_c.astype(np.float32)
        if rdx:
            out_c = _unpermute_rdx(out_c, c)
        outs.append(out_c)
    return np.concatenate(outs, axis=0)
